# revision 29
# baseline (speedup 1.0000x reference)
"""Trainium2 Bass kernel for quantized (AdaPT int8-systolic) 3x3 Conv2d.

Reference computation (see problem):
  amax_x = max(|x|) (global), amax_w = max(|w|)
  qx = clip(round(x * 127/amax_x)), qw likewise  (integer-valued)
  out = conv2d(qx, qw, pad=1) / ((127/amax_x)*(127/amax_w)) + bias

Sharding: batch N=32 -> 4 images per core across 8 cores (data parallel),
weight/bias replicated, amax_x via AllReduce-max collective.

Per-core layout: partition dim = (image, channel) = 4*32 = 128.

Single HBM pass over x: phase A streams x (f32), reduces |max| partials
and stores x as fp16 into a zero-padded resident [128, 226*226+2] image
(padding absorbs all conv edge effects in flat coordinates).  After the
amax AllReduce, quantization runs IN PLACE on the resident buffer
(x -> round(x*sx), still fp16: integers <= 127 are exact), so the conv
reads SBUF only.  fp16 keeps x to ~2^-11 relative error before
quantization (~0.3% output rel err, vs 2e-2 tolerance).

Conv = 9 accumulating matmuls per (image, 2-row tile): stationary
[32ci, 64co] fp16 per tap, moving = flat 226-px half-slices of the
padded image.  The free dim is split into two halves so each psum tile
gets 4 concurrent 32x64 PE sub-tiles (2 images x 2 row-halves) on
disjoint (row,col) positions; two psum tiles (2 image pairs) in flight
cover all 16 32x32 PE sub-arrays -> ~2x tensor-engine throughput vs
4-tile packing.  Accumulation in fp32 psum (< 2^24) is exact.

Output is written to HBM as bf16 (halves write traffic; ~0.1% rel err)
and upcast to f32 on the host.
"""

import os
import sys
import numpy as np
from contextlib import ExitStack

sys.path.insert(0, "/opt/trn_rl_repo")

MAGIC = 12582912.0  # 1.5 * 2^23: adding then subtracting forces RNE-to-int


def build(nimg=4, H=224, W=224, n_cores=8):
    import concourse.bass as bass
    import concourse.mybir as mybir
    import concourse.tile as tile
    from concourse import bacc
    from concourse import bass_isa

    f32 = mybir.dt.float32
    f16 = mybir.dt.float16
    bf16 = mybir.dt.bfloat16
    CI, CO = 32, 64
    HP, WP = H + 2, W + 2
    assert nimg == 4 and H % 2 == 0

    nc = bacc.Bacc()
    x_ext = nc.declare_dram_parameter("x", [nimg, CI, H, W], f32, isOutput=False)
    qw_ext = nc.declare_dram_parameter("qw_stat", [128, 9 * CO], f16,
                                       isOutput=False)
    sw_ext = nc.declare_dram_parameter("swv", [128, 1], f32, isOutput=False)
    b_ext = nc.declare_dram_parameter("bias", [CO], f32, isOutput=False)
    out_ext = nc.declare_dram_parameter("out", [nimg, CO, H, W], bf16,
                                        isOutput=True)

    cc_in = nc.dram_tensor("cc_in", [1, 1], f32)
    cc_out = nc.dram_tensor("cc_out", [1, 1], f32)

    AT = mybir.AluOpType
    AF = mybir.ActivationFunctionType

    with ExitStack() as ctx:
        tc = ctx.enter_context(tile.TileContext(nc))

        consts = ctx.enter_context(tc.tile_pool(name="consts", bufs=1))
        chunks = ctx.enter_context(tc.tile_pool(name="chunks", bufs=2))
        tmps = ctx.enter_context(tc.tile_pool(name="tmps", bufs=2))
        xhp = ctx.enter_context(tc.tile_pool(name="xhp", bufs=1))
        statp = ctx.enter_context(tc.tile_pool(name="statp", bufs=1))
        psump = ctx.enter_context(tc.tile_pool(name="psum", bufs=2, space="PSUM"))
        outsp = ctx.enter_context(tc.tile_pool(name="outs", bufs=3))

        # Warm up the collectives firmware with a dummy all-reduce so the
        # real amax all-reduce later isn't hit by one-time startup cost.
        if n_cores > 1:
            warm = consts.tile([1, 1], f32)
            nc.vector.memset(warm[:], 0.0)
            nc.sync.dma_start(cc_in[:, :], warm[:])
            nc.gpsimd.collective_compute(
                "AllReduce", AT.max,
                replica_groups=[list(range(n_cores))],
                ins=[cc_in[:, :].opt()],
                outs=[cc_out[:, :].opt()])

        # resident padded fp16 image; pads memset to 0 once, interior filled
        # by phase A.  (quantize(0)=0 so pads stay valid after in-place pass)
        xh = xhp.tile([128, HP * WP + 2], f16)
        xhv = xh[:, 0:HP * WP].rearrange("p (h w) -> p h w", w=WP)
        nc.vector.memset(xh[:, 0:WP], 0.0)                      # top pad row
        nc.vector.memset(xh[:, (HP - 1) * WP:HP * WP + 2], 0.0)  # bottom + tail
        nc.vector.memset(xhv[:, 1:HP - 1, 0:1], 0.0)             # left pad col
        nc.vector.memset(xhv[:, 1:HP - 1, WP - 1:WP], 0.0)       # right pad col

        # stationary weights: [ (4 image-groups x 32 ci) , (9 taps x 64 co) ]
        # quantized + transposed host-side; single contiguous DMA
        stat = statp.tile([128, 9 * CO], f16)
        nc.gpsimd.dma_start(stat[:], qw_ext[:, :])

        # ---------------- Phase A: stream x, amax partials + fp16 store -----
        xflat = x_ext[:, :, :, :].rearrange("n c h w -> (n c) (h w)")  # [128, H*W]
        RA = 16 if H % 16 == 0 else 2  # rows per streamed chunk
        n_amax_chunks = H // RA
        ce = RA * W
        partials = consts.tile([128, n_amax_chunks], f32)
        # PE warm-keeper: sparse dummy matmuls through phase A so the HAM
        # clock gate stays at 8/8 when the real conv matmuls begin. Each is
        # gated on its chunk's DMA so they spread through the phase.
        warm_ps = psump.tile([128, 512], f32, tag="ps")
        ones_row = consts.tile([1, 128], f32)
        nc.vector.memset(ones_row[:], 1.0)

        for k in range(n_amax_chunks):
            xt = chunks.tile([128, ce], f32, tag="chunk")
            ldeng = nc.gpsimd if k % 2 == 0 else nc.sync
            ldeng.dma_start(xt[:], xflat[:, k * ce:(k + 1) * ce])
            nc.vector.tensor_reduce(
                partials[:, k:k + 1], xt[:], axis=mybir.AxisListType.X,
                op=AT.max, apply_absolute_value=True)
            # fp16 store into the padded resident image
            nc.scalar.activation(
                xhv[:, k * RA + 1:(k + 1) * RA + 1, 1:W + 1],
                xt[:].rearrange("p (r w) -> p r w", w=W), AF.Copy)
            nc.tensor.matmul(warm_ps[:, 0:8], ones_row[:, :], xt[0:1, 0:8],
                             start=True, stop=True)

        amax_p = consts.tile([128, 1], f32)
        nc.vector.tensor_reduce(
            amax_p[:], partials[:], axis=mybir.AxisListType.X,
            op=AT.max, apply_absolute_value=True)
        # reduce across partitions (Pool-engine partition all-reduce)
        sc01 = consts.tile([128, 1], f32)
        nc.gpsimd.partition_all_reduce(
            sc01[:], amax_p[:], channels=128,
            reduce_op=bass_isa.ReduceOp.max)

        # global amax across cores via collective (cc_in written from the
        # same gpsimd queue that triggers the collective: no cross-engine hop)
        nc.gpsimd.dma_start(cc_in[:, :], sc01[0:1, 0:1])
        if n_cores > 1:
            nc.gpsimd.collective_compute(
                "AllReduce", AT.max,
                replica_groups=[list(range(n_cores))],
                ins=[cc_in[:, :].opt()],
                outs=[cc_out[:, :].opt()])
            cc_res = cc_out
        else:
            nc.gpsimd.dma_start(cc_out[:, :], cc_in[:, :])
            cc_res = cc_out
        gscal = consts.tile([128, 1], f32)  # p0: amax_x_global
        nc.sync.dma_start(gscal[0:1, 0:1], cc_res[:, :])

        # broadcast amax_x from partition 0 to all 128 partitions via a
        # K=1 matmul against a row of ones (standard instructions only)
        bc_ps = psump.tile([128, 1], f32, padded_shape=[128, 512], tag="ps")
        nc.tensor.matmul(bc_ps[:, :], ones_row[:, :], gscal[0:1, 0:1],
                         start=True, stop=True)
        # ---------------- scales --------------------------------------------
        rax = consts.tile([128, 1], f32)
        nc.vector.reciprocal(rax[:], bc_ps[:, :])
        sx = consts.tile([128, 1], f32)   # ~127/amax_x (1/amax then *127)
        nc.vector.tensor_scalar_mul(sx[:], rax[:], 127.0)
        sw = consts.tile([128, 1], f32)   # 127/amax_w (precomputed host-side)
        nc.gpsimd.dma_start(sw[:], sw_ext[:, :])
        scale2 = consts.tile([128, 1], f32)
        nc.vector.tensor_tensor(scale2[:], sx[:], sw[:], AT.mult)
        inv = consts.tile([128, 1], f32)  # 1/(sx*sw)
        nc.vector.reciprocal(inv[:], scale2[:])
        bias_vec = consts.tile([128, 1], f32)
        nc.gpsimd.dma_start(bias_vec[0:CO, :], b_ext[:].rearrange("(o u) -> o u", u=1))
        nc.gpsimd.dma_start(bias_vec[CO:2 * CO, :], b_ext[:].rearrange("(o u) -> o u", u=1))

        # ---------------- quantize x in place (fp16 -> fp16) ----------------
        # Two Scalar-engine passes per 8-row strip (MAGIC add then subtract
        # forces round-to-nearest-even to integer).
        RQ = 8 if H % 8 == 0 else 2
        n_q = H // RQ
        for rk in range(n_q):
            rows = xhv[:, rk * RQ + 1:(rk + 1) * RQ + 1, 1:W + 1]
            tmp = tmps.tile([128, RQ * W], f32)
            nc.scalar.activation(tmp[:].rearrange("p (r w) -> p r w", w=W),
                                 rows, AF.Copy, bias=MAGIC, scale=sx[:])
            # first strips' op2 on DVE so the quantize pipeline ramps at 2x
            # and the conv never starves at startup; DVE has slack until the
            # first epilogues arrive.
            op2e = nc.vector if rk < 6 else nc.scalar
            if op2e is nc.vector:
                op2e.tensor_scalar_add(
                    rows, tmp[:].rearrange("p (r w) -> p r w", w=W), -MAGIC)
            else:
                op2e.activation(rows,
                                tmp[:].rearrange("p (r w) -> p r w", w=W),
                                AF.Copy, bias=-MAGIC)

        # ---------------- conv: 9 taps, 4 output rows per super-iter --------
        # Super-iteration T covers output rows 4T..4T+3 (row pairs
        # t = 2T+u).  One 4-bank psum tile per T:
        #   img n, row pair u: PE tile pos (32n, 64u)
        #     -> ps[64u : 64u+64, 512n : 512n+452]   (full 452-col window
        #        = output rows 4T+2u, 4T+2u+1 in flat padded coords)
        # Each 32x64 PE tile owns its (bank, psum partition group)
        # exclusively (start=True resets the tile's whole partition group,
        # so two tiles must never share one).  8 tiles = all 16 sub-arrays,
        # and full-length windows amortize per-matmul fixed costs.
        # Epilogue: one scale+bias op per T over [128, 4, 2, 224] into an
        # SBUF staging ring (row pairs packed contiguously); the ring is
        # flushed every 8 super-iters as 8 large [64, 8, 448] DMAs
        # (DMA_DIRECT2D costs ~1us of engine time regardless of size).
        assert H % 4 == 0
        nT = H // 4
        SB = min(4, nT)          # super-iters per staging block
        assert nT % SB == 0
        CPI = 2 * W              # staging cols per image per super-iter
        CPT = nimg * CPI         # staging cols per super-iter
        # out rows viewed as (hb, uu, (par w)): row = 4*hb + 2*uu + par
        ov = out_ext[:, :, :, :].rearrange(
            "n o (hb uu par) w -> (n o) hb uu (par w)", uu=2, par=2)
        stg = None
        for T in range(nT):
            Ts = T % SB
            if Ts == 0:
                stg = outsp.tile([128, SB * CPT], bf16, tag="stg")
            ps = psump.tile([128, 4 * 512], f32, tag="ps")
            for tap in range(9):
                dy, dx = tap // 3, tap % 3
                for u in range(2):
                    off = (2 * (2 * T + u) + dy) * WP + dx
                    for n in range(nimg):
                        sa = stat[32 * n:32 * n + 32,
                                  tap * CO:(tap + 1) * CO]
                        nc.tensor.matmul(
                            ps[64 * u:64 * u + 64, 512 * n:512 * n + 2 * WP],
                            sa, xh[32 * n:32 * n + 32, off:off + 2 * WP],
                            start=(tap == 0), stop=(tap == 8),
                            skip_group_check=True,
                            tile_position=(32 * n, 64 * u))
            src = ps[:, :].rearrange("p (n c) -> p n c", n=4)[
                :, :, 0:2 * WP].rearrange(
                "p n (j w) -> p n j w", j=2)[:, :, :, 0:W]
            dst = stg[:, Ts * CPT:(Ts + 1) * CPT].rearrange(
                "p (n j w) -> p n j w", n=nimg, j=2)
            nc.vector.tensor_scalar(
                dst, src, inv[:], bias_vec[:], AT.mult, AT.add)
            if Ts == SB - 1:
                B = T // SB
                stgv = stg[:, :].rearrange("p (ts c) -> p ts c", ts=SB)
                for n in range(nimg):
                    for u in range(2):
                        eng = nc.sync if (n + u) % 2 == 0 else nc.gpsimd
                        eng.dma_start(
                            ov[n * CO:(n + 1) * CO,
                               SB * B:SB * B + SB, u:u + 1, 0:2 * W],
                            stgv[64 * u:64 * u + 64, :,
                                 n * CPI:(n + 1) * CPI])

    nc.finalize()
    return nc


def prep_weights(weight: np.ndarray) -> dict:
    """Host-side prep of the tiny replicated weight tensor: quantize
    (identical fp32 math to the reference) and lay out as the matmul
    stationary [(4 image-groups x 32 ci), (9 taps x 64 co)] in fp16."""
    w = weight.astype(np.float32)
    amax_w = np.float32(np.max(np.abs(w)))
    sw = np.float32(127.0) / amax_w
    qw = np.round(w * sw)  # RNE, matches jnp.round; |qw| <= 127 exact in fp16
    qs = np.transpose(qw.reshape(64, 32, 9), (1, 2, 0)).reshape(32, 576)
    qstat = np.ascontiguousarray(np.tile(qs, (4, 1))).astype(np.float16)
    swv = np.full((128, 1), sw, np.float32)
    return {"qw_stat": qstat, "swv": swv}


def kernel(x: np.ndarray, weight: np.ndarray, bias: np.ndarray) -> np.ndarray:
    from concourse.bass_utils import run_bass_kernel_spmd

    n_cores = 8
    N = x.shape[0]
    per = N // n_cores
    nc = build(nimg=per, H=x.shape[2], W=x.shape[3], n_cores=n_cores)
    wp = prep_weights(np.asarray(weight))
    in_maps = [
        {
            "x": np.ascontiguousarray(x[i * per:(i + 1) * per]),
            "qw_stat": wp["qw_stat"],
            "swv": wp["swv"],
            "bias": np.ascontiguousarray(bias),
        }
        for i in range(n_cores)
    ]
    res = run_bass_kernel_spmd(nc, in_maps, core_ids=list(range(n_cores)))
    outs = [np.asarray(r["out"]).astype(np.float32) for r in res.results]
    return np.concatenate(outs, axis=0)


if __name__ == "__main__":
    # smoke: tiny build only
    nc = build(nimg=4, H=8, W=8, n_cores=2)
    print("build ok")


# revision 31
# speedup vs baseline: 1.0491x; 1.0491x over previous
"""Trainium2 Bass kernel for quantized (AdaPT int8-systolic) 3x3 Conv2d.

Reference computation (see problem):
  amax_x = max(|x|) (global), amax_w = max(|w|)
  qx = clip(round(x * 127/amax_x)), qw likewise  (integer-valued)
  out = conv2d(qx, qw, pad=1) / ((127/amax_x)*(127/amax_w)) + bias

Sharding: batch N=32 -> 4 images per core across 8 cores (data parallel),
weight/bias replicated, amax_x via AllReduce-max collective.

Per-core layout: partition dim = (image, channel) = 4*32 = 128.

Single HBM pass over x: phase A streams x (f32), reduces |max| partials
and stores x as fp16 into a zero-padded resident [128, 226*226+2] image
(padding absorbs all conv edge effects in flat coordinates).  After the
amax AllReduce, quantization runs IN PLACE on the resident buffer
(x -> round(x*sx), still fp16: integers <= 127 are exact), so the conv
reads SBUF only.  fp16 keeps x to ~2^-11 relative error before
quantization (~0.3% output rel err, vs 2e-2 tolerance).

Conv = 9 accumulating matmuls per (image, 2-row tile): stationary
[32ci, 64co] fp16 per tap, moving = flat 226-px half-slices of the
padded image.  The free dim is split into two halves so each psum tile
gets 4 concurrent 32x64 PE sub-tiles (2 images x 2 row-halves) on
disjoint (row,col) positions; two psum tiles (2 image pairs) in flight
cover all 16 32x32 PE sub-arrays -> ~2x tensor-engine throughput vs
4-tile packing.  Accumulation in fp32 psum (< 2^24) is exact.

Output is written to HBM as bf16 (halves write traffic; ~0.1% rel err)
and upcast to f32 on the host.
"""

import os
import sys
import numpy as np
from contextlib import ExitStack

sys.path.insert(0, "/opt/trn_rl_repo")

MAGIC = 12582912.0  # 1.5 * 2^23: adding then subtracting forces RNE-to-int


def build(nimg=4, H=224, W=224, n_cores=8):
    import concourse.bass as bass
    import concourse.mybir as mybir
    import concourse.tile as tile
    from concourse import bacc
    from concourse import bass_isa

    f32 = mybir.dt.float32
    f16 = mybir.dt.float16
    bf16 = mybir.dt.bfloat16
    CI, CO = 32, 64
    HP, WP = H + 2, W + 2
    assert nimg == 4 and H % 2 == 0

    nc = bacc.Bacc()
    x_ext = nc.declare_dram_parameter("x", [nimg, CI, H, W], f32, isOutput=False)
    qw_ext = nc.declare_dram_parameter("qw_stat", [128, 9 * CO], f16,
                                       isOutput=False)
    sw_ext = nc.declare_dram_parameter("swv", [128, 1], f32, isOutput=False)
    b_ext = nc.declare_dram_parameter("bias", [CO], f32, isOutput=False)
    out_ext = nc.declare_dram_parameter("out", [nimg, CO, H, W], bf16,
                                        isOutput=True)

    cc_in = nc.dram_tensor("cc_in", [1, 1], f32)
    cc_out = nc.dram_tensor("cc_out", [1, 1], f32)

    AT = mybir.AluOpType
    AF = mybir.ActivationFunctionType

    with ExitStack() as ctx:
        tc = ctx.enter_context(tile.TileContext(nc))

        consts = ctx.enter_context(tc.tile_pool(name="consts", bufs=1))
        chunks = ctx.enter_context(tc.tile_pool(name="chunks", bufs=4))
        tmps = ctx.enter_context(tc.tile_pool(name="tmps", bufs=2))
        xhp = ctx.enter_context(tc.tile_pool(name="xhp", bufs=1))
        statp = ctx.enter_context(tc.tile_pool(name="statp", bufs=1))
        psump = ctx.enter_context(tc.tile_pool(name="psum", bufs=2, space="PSUM"))
        outsp = ctx.enter_context(tc.tile_pool(name="outs", bufs=3))

        # Warm up the collectives firmware with a dummy all-reduce so the
        # real amax all-reduce later isn't hit by one-time startup cost.
        if n_cores > 1:
            warm = consts.tile([1, 1], f32)
            nc.vector.memset(warm[:], 0.0)
            nc.sync.dma_start(cc_in[:, :], warm[:])
            nc.gpsimd.collective_compute(
                "AllReduce", AT.max,
                replica_groups=[list(range(n_cores))],
                ins=[cc_in[:, :].opt()],
                outs=[cc_out[:, :].opt()])

        # resident padded fp16 image; pads memset to 0 once, interior filled
        # by phase A.  (quantize(0)=0 so pads stay valid after in-place pass)
        xh = xhp.tile([128, HP * WP + 2], f16)
        xhv = xh[:, 0:HP * WP].rearrange("p (h w) -> p h w", w=WP)
        nc.vector.memset(xh[:, 0:WP], 0.0)                      # top pad row
        nc.vector.memset(xh[:, (HP - 1) * WP:HP * WP + 2], 0.0)  # bottom + tail
        nc.vector.memset(xhv[:, 1:HP - 1, 0:1], 0.0)             # left pad col
        nc.vector.memset(xhv[:, 1:HP - 1, WP - 1:WP], 0.0)       # right pad col

        # stationary weights: [ (4 image-groups x 32 ci) , (9 taps x 64 co) ]
        # quantized + transposed host-side; single contiguous DMA
        stat = statp.tile([128, 9 * CO], f16)
        nc.gpsimd.dma_start(stat[:], qw_ext[:, :])

        # ---------------- Phase A: stream x, amax partials + fp16 store -----
        xflat = x_ext[:, :, :, :].rearrange("n c h w -> (n c) (h w)")  # [128, H*W]
        RA = 8 if H % 8 == 0 else 2  # rows per streamed chunk
        n_amax_chunks = H // RA
        ce = RA * W
        partials = consts.tile([128, n_amax_chunks], f32)
        # PE warm-keeper: sparse dummy matmuls through phase A so the HAM
        # clock gate stays at 8/8 when the real conv matmuls begin. Each is
        # gated on its chunk's DMA so they spread through the phase.
        warm_ps = psump.tile([128, 512], f32, tag="ps")
        ones_row = consts.tile([1, 128], f32)
        nc.vector.memset(ones_row[:], 1.0)

        for k in range(n_amax_chunks):
            xt = chunks.tile([128, ce], f32, tag="chunk")
            ldeng = nc.gpsimd if k % 2 == 0 else nc.sync
            ldeng.dma_start(xt[:], xflat[:, k * ce:(k + 1) * ce])
            nc.vector.tensor_reduce(
                partials[:, k:k + 1], xt[:], axis=mybir.AxisListType.X,
                op=AT.max, apply_absolute_value=True)
            # fp16 store into the padded resident image
            nc.scalar.activation(
                xhv[:, k * RA + 1:(k + 1) * RA + 1, 1:W + 1],
                xt[:].rearrange("p (r w) -> p r w", w=W), AF.Copy)
            nc.tensor.matmul(warm_ps[:, 0:8], ones_row[:, :], xt[0:1, 0:8],
                             start=True, stop=True)

        amax_p = consts.tile([128, 1], f32)
        nc.vector.tensor_reduce(
            amax_p[:], partials[:], axis=mybir.AxisListType.X,
            op=AT.max, apply_absolute_value=True)
        # reduce across partitions (Pool-engine partition all-reduce)
        sc01 = consts.tile([128, 1], f32)
        nc.gpsimd.partition_all_reduce(
            sc01[:], amax_p[:], channels=128,
            reduce_op=bass_isa.ReduceOp.max)

        # global amax across cores via collective (cc_in written from the
        # same gpsimd queue that triggers the collective: no cross-engine hop)
        nc.gpsimd.dma_start(cc_in[:, :], sc01[0:1, 0:1])
        if n_cores > 1:
            nc.gpsimd.collective_compute(
                "AllReduce", AT.max,
                replica_groups=[list(range(n_cores))],
                ins=[cc_in[:, :].opt()],
                outs=[cc_out[:, :].opt()])
            cc_res = cc_out
        else:
            nc.gpsimd.dma_start(cc_out[:, :], cc_in[:, :])
            cc_res = cc_out
        gscal = consts.tile([128, 1], f32)  # p0: amax_x_global
        nc.sync.dma_start(gscal[0:1, 0:1], cc_res[:, :])

        # broadcast amax_x from partition 0 to all 128 partitions via a
        # K=1 matmul against a row of ones (standard instructions only)
        bc_ps = psump.tile([128, 1], f32, padded_shape=[128, 512], tag="ps")
        nc.tensor.matmul(bc_ps[:, :], ones_row[:, :], gscal[0:1, 0:1],
                         start=True, stop=True)
        # ---------------- scales --------------------------------------------
        rax = consts.tile([128, 1], f32)
        nc.vector.reciprocal(rax[:], bc_ps[:, :])
        sx = consts.tile([128, 1], f32)   # ~127/amax_x (1/amax then *127)
        nc.vector.tensor_scalar_mul(sx[:], rax[:], 127.0)
        sw = consts.tile([128, 1], f32)   # 127/amax_w (precomputed host-side)
        nc.gpsimd.dma_start(sw[:], sw_ext[:, :])
        scale2 = consts.tile([128, 1], f32)
        nc.vector.tensor_tensor(scale2[:], sx[:], sw[:], AT.mult)
        inv = consts.tile([128, 1], f32)  # 1/(sx*sw)
        nc.vector.reciprocal(inv[:], scale2[:])
        bias_vec = consts.tile([128, 1], f32)
        nc.gpsimd.dma_start(bias_vec[0:CO, :], b_ext[:].rearrange("(o u) -> o u", u=1))
        nc.gpsimd.dma_start(bias_vec[CO:2 * CO, :], b_ext[:].rearrange("(o u) -> o u", u=1))

        # ---------------- quantize x in place (fp16 -> fp16) ----------------
        # Two Scalar-engine passes per 8-row strip (MAGIC add then subtract
        # forces round-to-nearest-even to integer).
        RQ = 8 if H % 8 == 0 else 2
        n_q = H // RQ
        for rk in range(n_q):
            rows = xhv[:, rk * RQ + 1:(rk + 1) * RQ + 1, 1:W + 1]
            tmp = tmps.tile([128, RQ * W], f32)
            nc.scalar.activation(tmp[:].rearrange("p (r w) -> p r w", w=W),
                                 rows, AF.Copy, bias=MAGIC, scale=sx[:])
            # first strips' op2 on DVE so the quantize pipeline ramps at 2x
            # and the conv never starves at startup; DVE has slack until the
            # first epilogues arrive.
            op2e = nc.vector if rk < 6 else nc.scalar
            if op2e is nc.vector:
                op2e.tensor_scalar_add(
                    rows, tmp[:].rearrange("p (r w) -> p r w", w=W), -MAGIC)
            else:
                op2e.activation(rows,
                                tmp[:].rearrange("p (r w) -> p r w", w=W),
                                AF.Copy, bias=-MAGIC)

        # ---------------- conv: 9 taps, 4 output rows per super-iter --------
        # Super-iteration T covers output rows 4T..4T+3 (row pairs
        # t = 2T+u).  One 4-bank psum tile per T:
        #   img n, row pair u: PE tile pos (32n, 64u)
        #     -> ps[64u : 64u+64, 512n : 512n+452]   (full 452-col window
        #        = output rows 4T+2u, 4T+2u+1 in flat padded coords)
        # Each 32x64 PE tile owns its (bank, psum partition group)
        # exclusively (start=True resets the tile's whole partition group,
        # so two tiles must never share one).  8 tiles = all 16 sub-arrays,
        # and full-length windows amortize per-matmul fixed costs.
        # Epilogue: one scale+bias op per T over [128, 4, 2, 224] into an
        # SBUF staging ring (row pairs packed contiguously); the ring is
        # flushed every 8 super-iters as 8 large [64, 8, 448] DMAs
        # (DMA_DIRECT2D costs ~1us of engine time regardless of size).
        assert H % 4 == 0
        nT = H // 4
        SB = min(4, nT)          # super-iters per staging block
        assert nT % SB == 0
        CPI = 2 * W              # staging cols per image per super-iter
        CPT = nimg * CPI         # staging cols per super-iter
        # out rows viewed as (hb, uu, (par w)): row = 4*hb + 2*uu + par
        ov = out_ext[:, :, :, :].rearrange(
            "n o (hb uu par) w -> (n o) hb uu (par w)", uu=2, par=2)
        stg = None
        for T in range(nT):
            Ts = T % SB
            if Ts == 0:
                stg = outsp.tile([128, SB * CPT], bf16, tag="stg")
            ps = psump.tile([128, 4 * 512], f32, tag="ps")
            for tap in range(9):
                dy, dx = tap // 3, tap % 3
                for u in range(2):
                    off = (2 * (2 * T + u) + dy) * WP + dx
                    for n in range(nimg):
                        sa = stat[32 * n:32 * n + 32,
                                  tap * CO:(tap + 1) * CO]
                        nc.tensor.matmul(
                            ps[64 * u:64 * u + 64, 512 * n:512 * n + 2 * WP],
                            sa, xh[32 * n:32 * n + 32, off:off + 2 * WP],
                            start=(tap == 0), stop=(tap == 8),
                            skip_group_check=True,
                            tile_position=(32 * n, 64 * u))
            src = ps[:, :].rearrange("p (n c) -> p n c", n=4)[
                :, :, 0:2 * WP].rearrange(
                "p n (j w) -> p n j w", j=2)[:, :, :, 0:W]
            dst = stg[:, Ts * CPT:(Ts + 1) * CPT].rearrange(
                "p (n j w) -> p n j w", n=nimg, j=2)
            nc.vector.tensor_scalar(
                dst, src, inv[:], bias_vec[:], AT.mult, AT.add)
            if Ts == SB - 1:
                B = T // SB
                stgv = stg[:, :].rearrange("p (ts c) -> p ts c", ts=SB)
                for n in range(nimg):
                    for u in range(2):
                        eng = nc.sync if (n + u) % 2 == 0 else nc.gpsimd
                        eng.dma_start(
                            ov[n * CO:(n + 1) * CO,
                               SB * B:SB * B + SB, u:u + 1, 0:2 * W],
                            stgv[64 * u:64 * u + 64, :,
                                 n * CPI:(n + 1) * CPI])

    nc.finalize()
    return nc


def prep_weights(weight: np.ndarray) -> dict:
    """Host-side prep of the tiny replicated weight tensor: quantize
    (identical fp32 math to the reference) and lay out as the matmul
    stationary [(4 image-groups x 32 ci), (9 taps x 64 co)] in fp16."""
    w = weight.astype(np.float32)
    amax_w = np.float32(np.max(np.abs(w)))
    sw = np.float32(127.0) / amax_w
    qw = np.round(w * sw)  # RNE, matches jnp.round; |qw| <= 127 exact in fp16
    qs = np.transpose(qw.reshape(64, 32, 9), (1, 2, 0)).reshape(32, 576)
    qstat = np.ascontiguousarray(np.tile(qs, (4, 1))).astype(np.float16)
    swv = np.full((128, 1), sw, np.float32)
    return {"qw_stat": qstat, "swv": swv}


def kernel(x: np.ndarray, weight: np.ndarray, bias: np.ndarray) -> np.ndarray:
    from concourse.bass_utils import run_bass_kernel_spmd

    n_cores = 8
    N = x.shape[0]
    per = N // n_cores
    nc = build(nimg=per, H=x.shape[2], W=x.shape[3], n_cores=n_cores)
    wp = prep_weights(np.asarray(weight))
    in_maps = [
        {
            "x": np.ascontiguousarray(x[i * per:(i + 1) * per]),
            "qw_stat": wp["qw_stat"],
            "swv": wp["swv"],
            "bias": np.ascontiguousarray(bias),
        }
        for i in range(n_cores)
    ]
    res = run_bass_kernel_spmd(nc, in_maps, core_ids=list(range(n_cores)))
    outs = [np.asarray(r["out"]).astype(np.float32) for r in res.results]
    return np.concatenate(outs, axis=0)


if __name__ == "__main__":
    # smoke: tiny build only
    nc = build(nimg=4, H=8, W=8, n_cores=2)
    print("build ok")


# revision 32
# speedup vs baseline: 1.0493x; 1.0002x over previous
"""Trainium2 Bass kernel for quantized (AdaPT int8-systolic) 3x3 Conv2d.

Reference computation (see problem):
  amax_x = max(|x|) (global), amax_w = max(|w|)
  qx = clip(round(x * 127/amax_x)), qw likewise  (integer-valued)
  out = conv2d(qx, qw, pad=1) / ((127/amax_x)*(127/amax_w)) + bias

Sharding: batch N=32 -> 4 images per core across 8 cores (data parallel),
weight/bias replicated, amax_x via AllReduce-max collective.

Per-core layout: partition dim = (image, channel) = 4*32 = 128.

Single HBM pass over x: phase A streams x (f32), reduces |max| partials
and stores x as fp16 into a zero-padded resident [128, 226*226+2] image
(padding absorbs all conv edge effects in flat coordinates).  After the
amax AllReduce, quantization runs IN PLACE on the resident buffer
(x -> round(x*sx), still fp16: integers <= 127 are exact), so the conv
reads SBUF only.  fp16 keeps x to ~2^-11 relative error before
quantization (~0.3% output rel err, vs 2e-2 tolerance).

Conv = 9 accumulating matmuls per (image, 2-row tile): stationary
[32ci, 64co] fp16 per tap, moving = flat 226-px half-slices of the
padded image.  The free dim is split into two halves so each psum tile
gets 4 concurrent 32x64 PE sub-tiles (2 images x 2 row-halves) on
disjoint (row,col) positions; two psum tiles (2 image pairs) in flight
cover all 16 32x32 PE sub-arrays -> ~2x tensor-engine throughput vs
4-tile packing.  Accumulation in fp32 psum (< 2^24) is exact.

Output is written to HBM as bf16 (halves write traffic; ~0.1% rel err)
and upcast to f32 on the host.
"""

import os
import sys
import numpy as np
from contextlib import ExitStack

sys.path.insert(0, "/opt/trn_rl_repo")

MAGIC = 12582912.0  # 1.5 * 2^23: adding then subtracting forces RNE-to-int


def build(nimg=4, H=224, W=224, n_cores=8):
    import concourse.bass as bass
    import concourse.mybir as mybir
    import concourse.tile as tile
    from concourse import bacc
    from concourse import bass_isa

    f32 = mybir.dt.float32
    f16 = mybir.dt.float16
    bf16 = mybir.dt.bfloat16
    CI, CO = 32, 64
    HP, WP = H + 2, W + 2
    assert nimg == 4 and H % 2 == 0

    nc = bacc.Bacc()
    x_ext = nc.declare_dram_parameter("x", [nimg, CI, H, W], f32, isOutput=False)
    qw_ext = nc.declare_dram_parameter("qw_stat", [128, 9 * CO], f16,
                                       isOutput=False)
    sw_ext = nc.declare_dram_parameter("swv", [128, 1], f32, isOutput=False)
    b_ext = nc.declare_dram_parameter("bias", [CO], f32, isOutput=False)
    out_ext = nc.declare_dram_parameter("out", [nimg, CO, H, W], bf16,
                                        isOutput=True)

    cc_in = nc.dram_tensor("cc_in", [1, 1], f32)
    cc_out = nc.dram_tensor("cc_out", [1, 1], f32)

    AT = mybir.AluOpType
    AF = mybir.ActivationFunctionType

    with ExitStack() as ctx:
        tc = ctx.enter_context(tile.TileContext(nc))

        consts = ctx.enter_context(tc.tile_pool(name="consts", bufs=1))
        chunks = ctx.enter_context(tc.tile_pool(name="chunks", bufs=4))
        tmps = ctx.enter_context(tc.tile_pool(name="tmps", bufs=2))
        xhp = ctx.enter_context(tc.tile_pool(name="xhp", bufs=1))
        statp = ctx.enter_context(tc.tile_pool(name="statp", bufs=1))
        psump = ctx.enter_context(tc.tile_pool(name="psum", bufs=2, space="PSUM"))
        outsp = ctx.enter_context(tc.tile_pool(name="outs", bufs=3))

        # Warm up the collectives firmware with a dummy all-reduce so the
        # real amax all-reduce later isn't hit by one-time startup cost.
        if n_cores > 1:
            warm = consts.tile([1, 1], f32)
            nc.vector.memset(warm[:], 0.0)
            nc.sync.dma_start(cc_in[:, :], warm[:])
            nc.gpsimd.collective_compute(
                "AllReduce", AT.max,
                replica_groups=[list(range(n_cores))],
                ins=[cc_in[:, :].opt()],
                outs=[cc_out[:, :].opt()])

        # resident padded fp16 image; pads memset to 0 once, interior filled
        # by phase A.  (quantize(0)=0 so pads stay valid after in-place pass)
        xh = xhp.tile([128, HP * WP + 2], f16)
        xhv = xh[:, 0:HP * WP].rearrange("p (h w) -> p h w", w=WP)
        nc.vector.memset(xh[:, 0:WP], 0.0)                      # top pad row
        nc.vector.memset(xh[:, (HP - 1) * WP:HP * WP + 2], 0.0)  # bottom + tail
        nc.vector.memset(xhv[:, 1:HP - 1, 0:1], 0.0)             # left pad col
        nc.vector.memset(xhv[:, 1:HP - 1, WP - 1:WP], 0.0)       # right pad col

        # stationary weights: [ (4 image-groups x 32 ci) , (9 taps x 64 co) ]
        # quantized + transposed host-side; single contiguous DMA
        stat = statp.tile([128, 9 * CO], f16)
        nc.gpsimd.dma_start(stat[:], qw_ext[:, :])

        # ---------------- Phase A: stream x, amax partials + fp16 store -----
        xflat = x_ext[:, :, :, :].rearrange("n c h w -> (n c) (h w)")  # [128, H*W]
        RA = 8 if H % 8 == 0 else 2  # rows per streamed chunk
        n_amax_chunks = H // RA
        ce = RA * W
        partials = consts.tile([128, n_amax_chunks], f32)
        # PE warm-keeper: sparse dummy matmuls through phase A so the HAM
        # clock gate stays at 8/8 when the real conv matmuls begin. Each is
        # gated on its chunk's DMA so they spread through the phase.
        warm_ps = psump.tile([128, 512], f32, tag="ps")
        ones_row = consts.tile([1, 128], f32)
        nc.vector.memset(ones_row[:], 1.0)

        for k in range(n_amax_chunks):
            xt = chunks.tile([128, ce], f32, tag="chunk")
            ldeng = nc.gpsimd if k % 2 == 0 else nc.sync
            ldeng.dma_start(xt[:], xflat[:, k * ce:(k + 1) * ce])
            nc.vector.tensor_reduce(
                partials[:, k:k + 1], xt[:], axis=mybir.AxisListType.X,
                op=AT.max, apply_absolute_value=True)
            # fp16 store into the padded resident image
            nc.scalar.activation(
                xhv[:, k * RA + 1:(k + 1) * RA + 1, 1:W + 1],
                xt[:].rearrange("p (r w) -> p r w", w=W), AF.Copy)
            nc.tensor.matmul(warm_ps[:, 0:8], ones_row[:, :], xt[0:1, 0:8],
                             start=True, stop=True)

        amax_p = consts.tile([128, 1], f32)
        nc.vector.tensor_reduce(
            amax_p[:], partials[:], axis=mybir.AxisListType.X,
            op=AT.max, apply_absolute_value=True)
        # reduce across partitions (Pool-engine partition all-reduce)
        sc01 = consts.tile([128, 1], f32)
        nc.gpsimd.partition_all_reduce(
            sc01[:], amax_p[:], channels=128,
            reduce_op=bass_isa.ReduceOp.max)

        # global amax across cores via collective (cc_in written from the
        # same gpsimd queue that triggers the collective: no cross-engine hop)
        nc.gpsimd.dma_start(cc_in[:, :], sc01[0:1, 0:1])
        if n_cores > 1:
            nc.gpsimd.collective_compute(
                "AllReduce", AT.max,
                replica_groups=[list(range(n_cores))],
                ins=[cc_in[:, :].opt()],
                outs=[cc_out[:, :].opt()])
            cc_res = cc_out
        else:
            nc.gpsimd.dma_start(cc_out[:, :], cc_in[:, :])
            cc_res = cc_out
        gscal = consts.tile([128, 1], f32)  # p0: amax_x_global
        nc.sync.dma_start(gscal[0:1, 0:1], cc_res[:, :])

        # broadcast amax_x from partition 0 to all 128 partitions via a
        # K=1 matmul against a row of ones (standard instructions only)
        bc_ps = psump.tile([128, 1], f32, padded_shape=[128, 512], tag="ps")
        nc.tensor.matmul(bc_ps[:, :], ones_row[:, :], gscal[0:1, 0:1],
                         start=True, stop=True)
        # ---------------- scales --------------------------------------------
        rax = consts.tile([128, 1], f32)
        nc.vector.reciprocal(rax[:], bc_ps[:, :])
        sx = consts.tile([128, 1], f32)   # ~127/amax_x (1/amax then *127)
        nc.vector.tensor_scalar_mul(sx[:], rax[:], 127.0)
        sw = consts.tile([128, 1], f32)   # 127/amax_w (precomputed host-side)
        nc.gpsimd.dma_start(sw[:], sw_ext[:, :])
        scale2 = consts.tile([128, 1], f32)
        nc.vector.tensor_tensor(scale2[:], sx[:], sw[:], AT.mult)
        inv = consts.tile([128, 1], f32)  # 1/(sx*sw)
        nc.vector.reciprocal(inv[:], scale2[:])
        bias_vec = consts.tile([128, 1], f32)
        nc.gpsimd.dma_start(bias_vec[0:CO, :], b_ext[:].rearrange("(o u) -> o u", u=1))
        nc.gpsimd.dma_start(bias_vec[CO:2 * CO, :], b_ext[:].rearrange("(o u) -> o u", u=1))

        # ---------------- quantize x in place (fp16 -> fp16) ----------------
        # Two Scalar-engine passes per 8-row strip (MAGIC add then subtract
        # forces round-to-nearest-even to integer).
        RQ = 8 if H % 8 == 0 else 2
        n_q = H // RQ
        for rk in range(n_q):
            rows = xhv[:, rk * RQ + 1:(rk + 1) * RQ + 1, 1:W + 1]
            tmp = tmps.tile([128, RQ * W], f32)
            nc.scalar.activation(tmp[:].rearrange("p (r w) -> p r w", w=W),
                                 rows, AF.Copy, bias=MAGIC, scale=sx[:])
            nc.scalar.activation(rows,
                                 tmp[:].rearrange("p (r w) -> p r w", w=W),
                                 AF.Copy, bias=-MAGIC)

        # ---------------- conv: 9 taps, 4 output rows per super-iter --------
        # Super-iteration T covers output rows 4T..4T+3 (row pairs
        # t = 2T+u).  One 4-bank psum tile per T:
        #   img n, row pair u: PE tile pos (32n, 64u)
        #     -> ps[64u : 64u+64, 512n : 512n+452]   (full 452-col window
        #        = output rows 4T+2u, 4T+2u+1 in flat padded coords)
        # Each 32x64 PE tile owns its (bank, psum partition group)
        # exclusively (start=True resets the tile's whole partition group,
        # so two tiles must never share one).  8 tiles = all 16 sub-arrays,
        # and full-length windows amortize per-matmul fixed costs.
        # Epilogue: one scale+bias op per T over [128, 4, 2, 224] into an
        # SBUF staging ring (row pairs packed contiguously); the ring is
        # flushed every 8 super-iters as 8 large [64, 8, 448] DMAs
        # (DMA_DIRECT2D costs ~1us of engine time regardless of size).
        assert H % 4 == 0
        nT = H // 4
        SB = min(4, nT)          # super-iters per staging block
        assert nT % SB == 0
        CPI = 2 * W              # staging cols per image per super-iter
        CPT = nimg * CPI         # staging cols per super-iter
        # out rows viewed as (hb, uu, (par w)): row = 4*hb + 2*uu + par
        ov = out_ext[:, :, :, :].rearrange(
            "n o (hb uu par) w -> (n o) hb uu (par w)", uu=2, par=2)
        stg = None
        for T in range(nT):
            Ts = T % SB
            if Ts == 0:
                stg = outsp.tile([128, SB * CPT], bf16, tag="stg")
            ps = psump.tile([128, 4 * 512], f32, tag="ps")
            for tap in range(9):
                dy, dx = tap // 3, tap % 3
                for u in range(2):
                    off = (2 * (2 * T + u) + dy) * WP + dx
                    for n in range(nimg):
                        sa = stat[32 * n:32 * n + 32,
                                  tap * CO:(tap + 1) * CO]
                        nc.tensor.matmul(
                            ps[64 * u:64 * u + 64, 512 * n:512 * n + 2 * WP],
                            sa, xh[32 * n:32 * n + 32, off:off + 2 * WP],
                            start=(tap == 0), stop=(tap == 8),
                            skip_group_check=True,
                            tile_position=(32 * n, 64 * u))
            src = ps[:, :].rearrange("p (n c) -> p n c", n=4)[
                :, :, 0:2 * WP].rearrange(
                "p n (j w) -> p n j w", j=2)[:, :, :, 0:W]
            dst = stg[:, Ts * CPT:(Ts + 1) * CPT].rearrange(
                "p (n j w) -> p n j w", n=nimg, j=2)
            nc.vector.tensor_scalar(
                dst, src, inv[:], bias_vec[:], AT.mult, AT.add)
            if Ts == SB - 1:
                B = T // SB
                stgv = stg[:, :].rearrange("p (ts c) -> p ts c", ts=SB)
                for n in range(nimg):
                    for u in range(2):
                        eng = nc.sync if (n + u) % 2 == 0 else nc.gpsimd
                        eng.dma_start(
                            ov[n * CO:(n + 1) * CO,
                               SB * B:SB * B + SB, u:u + 1, 0:2 * W],
                            stgv[64 * u:64 * u + 64, :,
                                 n * CPI:(n + 1) * CPI])

    nc.finalize()
    return nc


def prep_weights(weight: np.ndarray) -> dict:
    """Host-side prep of the tiny replicated weight tensor: quantize
    (identical fp32 math to the reference) and lay out as the matmul
    stationary [(4 image-groups x 32 ci), (9 taps x 64 co)] in fp16."""
    w = weight.astype(np.float32)
    amax_w = np.float32(np.max(np.abs(w)))
    sw = np.float32(127.0) / amax_w
    qw = np.round(w * sw)  # RNE, matches jnp.round; |qw| <= 127 exact in fp16
    qs = np.transpose(qw.reshape(64, 32, 9), (1, 2, 0)).reshape(32, 576)
    qstat = np.ascontiguousarray(np.tile(qs, (4, 1))).astype(np.float16)
    swv = np.full((128, 1), sw, np.float32)
    return {"qw_stat": qstat, "swv": swv}


def kernel(x: np.ndarray, weight: np.ndarray, bias: np.ndarray) -> np.ndarray:
    from concourse.bass_utils import run_bass_kernel_spmd

    n_cores = 8
    N = x.shape[0]
    per = N // n_cores
    nc = build(nimg=per, H=x.shape[2], W=x.shape[3], n_cores=n_cores)
    wp = prep_weights(np.asarray(weight))
    in_maps = [
        {
            "x": np.ascontiguousarray(x[i * per:(i + 1) * per]),
            "qw_stat": wp["qw_stat"],
            "swv": wp["swv"],
            "bias": np.ascontiguousarray(bias),
        }
        for i in range(n_cores)
    ]
    res = run_bass_kernel_spmd(nc, in_maps, core_ids=list(range(n_cores)))
    outs = [np.asarray(r["out"]).astype(np.float32) for r in res.results]
    return np.concatenate(outs, axis=0)


if __name__ == "__main__":
    # smoke: tiny build only
    nc = build(nimg=4, H=8, W=8, n_cores=2)
    print("build ok")


# revision 34
# speedup vs baseline: 1.1047x; 1.0529x over previous
"""Trainium2 Bass kernel for quantized (AdaPT int8-systolic) 3x3 Conv2d.

Reference computation (see problem):
  amax_x = max(|x|) (global), amax_w = max(|w|)
  qx = clip(round(x * 127/amax_x)), qw likewise  (integer-valued)
  out = conv2d(qx, qw, pad=1) / ((127/amax_x)*(127/amax_w)) + bias

Sharding: batch N=32 -> 4 images per core across 8 cores (data parallel),
weight/bias replicated, amax_x via AllReduce-max collective.

Per-core layout: partition dim = (image, channel) = 4*32 = 128.

Single HBM pass over x: phase A streams x (f32), reduces |max| partials
and stores x as fp16 into a zero-padded resident [128, 226*226+2] image
(padding absorbs all conv edge effects in flat coordinates).  After the
amax AllReduce, quantization runs IN PLACE on the resident buffer
(x -> round(x*sx), still fp16: integers <= 127 are exact), so the conv
reads SBUF only.  fp16 keeps x to ~2^-11 relative error before
quantization (~0.3% output rel err, vs 2e-2 tolerance).

Conv = 9 accumulating matmuls per (image, 2-row tile): stationary
[32ci, 64co] fp16 per tap, moving = flat 226-px half-slices of the
padded image.  The free dim is split into two halves so each psum tile
gets 4 concurrent 32x64 PE sub-tiles (2 images x 2 row-halves) on
disjoint (row,col) positions; two psum tiles (2 image pairs) in flight
cover all 16 32x32 PE sub-arrays -> ~2x tensor-engine throughput vs
4-tile packing.  Accumulation in fp32 psum (< 2^24) is exact.

Output is written to HBM as bf16 (halves write traffic; ~0.1% rel err)
and upcast to f32 on the host.
"""

import os
import sys
import numpy as np
from contextlib import ExitStack

sys.path.insert(0, "/opt/trn_rl_repo")

MAGIC = 12582912.0  # 1.5 * 2^23: adding then subtracting forces RNE-to-int


def build(nimg=4, H=224, W=224, n_cores=8):
    import concourse.bass as bass
    import concourse.mybir as mybir
    import concourse.tile as tile
    from concourse import bacc
    from concourse import bass_isa

    f32 = mybir.dt.float32
    f16 = mybir.dt.float16
    bf16 = mybir.dt.bfloat16
    CI, CO = 32, 64
    HP, WP = H + 2, W + 2
    assert nimg == 4 and H % 2 == 0

    nc = bacc.Bacc()
    x_ext = nc.declare_dram_parameter("x", [nimg, CI, H, W], f32, isOutput=False)
    qw_ext = nc.declare_dram_parameter("qw_stat", [128, 9 * CO], f16,
                                       isOutput=False)
    sw_ext = nc.declare_dram_parameter("swv", [128, 1], f32, isOutput=False)
    b_ext = nc.declare_dram_parameter("bias", [CO], f32, isOutput=False)
    out_ext = nc.declare_dram_parameter("out", [nimg, CO, H, W], bf16,
                                        isOutput=True)

    cc_in = nc.dram_tensor("cc_in", [1, 1], f32)
    cc_out = nc.dram_tensor("cc_out", [1, 1], f32)

    AT = mybir.AluOpType
    AF = mybir.ActivationFunctionType

    with ExitStack() as ctx:
        tc = ctx.enter_context(tile.TileContext(nc))

        consts = ctx.enter_context(tc.tile_pool(name="consts", bufs=1))
        chunks = ctx.enter_context(tc.tile_pool(name="chunks", bufs=4))
        tmps = ctx.enter_context(tc.tile_pool(name="tmps", bufs=1))
        xhp = ctx.enter_context(tc.tile_pool(name="xhp", bufs=1))
        statp = ctx.enter_context(tc.tile_pool(name="statp", bufs=1))
        psump = ctx.enter_context(tc.tile_pool(name="psum", bufs=2, space="PSUM"))
        outsp = ctx.enter_context(tc.tile_pool(name="outs", bufs=4))

        # Warm up the collectives firmware with a dummy all-reduce so the
        # real amax all-reduce later isn't hit by one-time startup cost.
        if n_cores > 1:
            warm = consts.tile([1, 1], f32)
            nc.vector.memset(warm[:], 0.0)
            nc.sync.dma_start(cc_in[:, :], warm[:])
            nc.gpsimd.collective_compute(
                "AllReduce", AT.max,
                replica_groups=[list(range(n_cores))],
                ins=[cc_in[:, :].opt()],
                outs=[cc_out[:, :].opt()])

        # resident padded fp16 image; pads memset to 0 once, interior filled
        # by phase A.  (quantize(0)=0 so pads stay valid after in-place pass)
        xh = xhp.tile([128, HP * WP + 2], f16)
        xhv = xh[:, 0:HP * WP].rearrange("p (h w) -> p h w", w=WP)
        nc.vector.memset(xh[:, 0:WP], 0.0)                      # top pad row
        nc.vector.memset(xh[:, (HP - 1) * WP:HP * WP + 2], 0.0)  # bottom + tail
        nc.vector.memset(xhv[:, 1:HP - 1, 0:1], 0.0)             # left pad col
        nc.vector.memset(xhv[:, 1:HP - 1, WP - 1:WP], 0.0)       # right pad col

        # stationary weights: [ (4 image-groups x 32 ci) , (9 taps x 64 co) ]
        # quantized + transposed host-side; single contiguous DMA
        stat = statp.tile([128, 9 * CO], f16)
        nc.gpsimd.dma_start(stat[:], qw_ext[:, :])

        # ---------------- Phase A: stream x, amax partials + fp16 store -----
        xflat = x_ext[:, :, :, :].rearrange("n c h w -> (n c) (h w)")  # [128, H*W]
        RA = 8 if H % 8 == 0 else 2  # rows per streamed chunk
        n_amax_chunks = H // RA
        ce = RA * W
        partials = consts.tile([128, n_amax_chunks], f32)
        # PE warm-keeper: sparse dummy matmuls through phase A so the HAM
        # clock gate stays at 8/8 when the real conv matmuls begin. Each is
        # gated on its chunk's DMA so they spread through the phase.
        warm_ps = psump.tile([128, 512], f32, tag="ps")
        ones_row = consts.tile([1, 128], f32)
        nc.vector.memset(ones_row[:], 1.0)

        for k in range(n_amax_chunks):
            xt = chunks.tile([128, ce], f32, tag="chunk")
            ldeng = nc.gpsimd if k % 2 == 0 else nc.sync
            ldeng.dma_start(xt[:], xflat[:, k * ce:(k + 1) * ce])
            nc.vector.tensor_reduce(
                partials[:, k:k + 1], xt[:], axis=mybir.AxisListType.X,
                op=AT.max, apply_absolute_value=True)
            # fp16 store into the padded resident image
            nc.scalar.activation(
                xhv[:, k * RA + 1:(k + 1) * RA + 1, 1:W + 1],
                xt[:].rearrange("p (r w) -> p r w", w=W), AF.Copy)
            nc.tensor.matmul(warm_ps[:, 0:8], ones_row[:, :], xt[0:1, 0:8],
                             start=True, stop=True)

        amax_p = consts.tile([128, 1], f32)
        nc.vector.tensor_reduce(
            amax_p[:], partials[:], axis=mybir.AxisListType.X,
            op=AT.max, apply_absolute_value=True)
        # reduce across partitions (Pool-engine partition all-reduce)
        sc01 = consts.tile([128, 1], f32)
        nc.gpsimd.partition_all_reduce(
            sc01[:], amax_p[:], channels=128,
            reduce_op=bass_isa.ReduceOp.max)

        # global amax across cores via collective (cc_in written from the
        # same gpsimd queue that triggers the collective: no cross-engine hop)
        nc.gpsimd.dma_start(cc_in[:, :], sc01[0:1, 0:1])
        if n_cores > 1:
            nc.gpsimd.collective_compute(
                "AllReduce", AT.max,
                replica_groups=[list(range(n_cores))],
                ins=[cc_in[:, :].opt()],
                outs=[cc_out[:, :].opt()])
            cc_res = cc_out
        else:
            nc.gpsimd.dma_start(cc_out[:, :], cc_in[:, :])
            cc_res = cc_out
        gscal = consts.tile([128, 1], f32)  # p0: amax_x_global
        nc.sync.dma_start(gscal[0:1, 0:1], cc_res[:, :])

        # broadcast amax_x from partition 0 to all 128 partitions via a
        # K=1 matmul against a row of ones (standard instructions only)
        bc_ps = psump.tile([128, 1], f32, padded_shape=[128, 512], tag="ps")
        nc.tensor.matmul(bc_ps[:, :], ones_row[:, :], gscal[0:1, 0:1],
                         start=True, stop=True)
        # ---------------- scales --------------------------------------------
        rax = consts.tile([128, 1], f32)
        nc.vector.reciprocal(rax[:], bc_ps[:, :])
        sx = consts.tile([128, 1], f32)   # ~127/amax_x (1/amax then *127)
        nc.vector.tensor_scalar_mul(sx[:], rax[:], 127.0)
        sw = consts.tile([128, 1], f32)   # 127/amax_w (precomputed host-side)
        nc.gpsimd.dma_start(sw[:], sw_ext[:, :])
        scale2 = consts.tile([128, 1], f32)
        nc.vector.tensor_tensor(scale2[:], sx[:], sw[:], AT.mult)
        inv = consts.tile([128, 1], f32)  # 1/(sx*sw)
        nc.vector.reciprocal(inv[:], scale2[:])
        bias_vec = consts.tile([128, 1], f32)
        nc.gpsimd.dma_start(bias_vec[0:CO, :], b_ext[:].rearrange("(o u) -> o u", u=1))
        nc.gpsimd.dma_start(bias_vec[CO:2 * CO, :], b_ext[:].rearrange("(o u) -> o u", u=1))

        # ---------------- quantize x in place (fp16 -> fp16) ----------------
        # Two Scalar-engine passes per 8-row strip (MAGIC add then subtract
        # forces round-to-nearest-even to integer).
        RQ = 8 if H % 8 == 0 else 2
        n_q = H // RQ
        for rk in range(n_q):
            rows = xhv[:, rk * RQ + 1:(rk + 1) * RQ + 1, 1:W + 1]
            tmp = tmps.tile([128, RQ * W], f32)
            nc.scalar.activation(tmp[:].rearrange("p (r w) -> p r w", w=W),
                                 rows, AF.Copy, bias=MAGIC, scale=sx[:])
            nc.scalar.activation(rows,
                                 tmp[:].rearrange("p (r w) -> p r w", w=W),
                                 AF.Copy, bias=-MAGIC)

        # ---------------- conv: 9 taps, 4 output rows per super-iter --------
        # Super-iteration T covers output rows 4T..4T+3 (row pairs
        # t = 2T+u).  One 4-bank psum tile per T:
        #   img n, row pair u: PE tile pos (32n, 64u)
        #     -> ps[64u : 64u+64, 512n : 512n+452]   (full 452-col window
        #        = output rows 4T+2u, 4T+2u+1 in flat padded coords)
        # Each 32x64 PE tile owns its (bank, psum partition group)
        # exclusively (start=True resets the tile's whole partition group,
        # so two tiles must never share one).  8 tiles = all 16 sub-arrays,
        # and full-length windows amortize per-matmul fixed costs.
        # Epilogue: one scale+bias op per T over [128, 4, 2, 224] into an
        # SBUF staging ring (row pairs packed contiguously); the ring is
        # flushed every 8 super-iters as 8 large [64, 8, 448] DMAs
        # (DMA_DIRECT2D costs ~1us of engine time regardless of size).
        assert H % 4 == 0
        nT = H // 4
        SB = min(4, nT)          # super-iters per staging block
        assert nT % SB == 0
        CPI = 2 * W              # staging cols per image per super-iter
        CPT = nimg * CPI         # staging cols per super-iter
        # out rows viewed as (hb, uu, (par w)): row = 4*hb + 2*uu + par
        ov = out_ext[:, :, :, :].rearrange(
            "n o (hb uu par) w -> (n o) hb uu (par w)", uu=2, par=2)
        stg = None
        for T in range(nT):
            Ts = T % SB
            if Ts == 0:
                stg = outsp.tile([128, SB * CPT], bf16, tag="stg")
            ps = psump.tile([128, 4 * 512], f32, tag="ps")
            for tap in range(9):
                dy, dx = tap // 3, tap % 3
                for u in range(2):
                    off = (2 * (2 * T + u) + dy) * WP + dx
                    for n in range(nimg):
                        sa = stat[32 * n:32 * n + 32,
                                  tap * CO:(tap + 1) * CO]
                        nc.tensor.matmul(
                            ps[64 * u:64 * u + 64, 512 * n:512 * n + 2 * WP],
                            sa, xh[32 * n:32 * n + 32, off:off + 2 * WP],
                            start=(tap == 0), stop=(tap == 8),
                            skip_group_check=True,
                            tile_position=(32 * n, 64 * u))
            src = ps[:, :].rearrange("p (n c) -> p n c", n=4)[
                :, :, 0:2 * WP].rearrange(
                "p n (j w) -> p n j w", j=2)[:, :, :, 0:W]
            dst = stg[:, Ts * CPT:(Ts + 1) * CPT].rearrange(
                "p (n j w) -> p n j w", n=nimg, j=2)
            nc.vector.tensor_scalar(
                dst, src, inv[:], bias_vec[:], AT.mult, AT.add)
            if Ts == SB - 1:
                B = T // SB
                stgv = stg[:, :].rearrange("p (ts c) -> p ts c", ts=SB)
                for n in range(nimg):
                    for u in range(2):
                        eng = nc.sync if (n + u) % 2 == 0 else nc.gpsimd
                        eng.dma_start(
                            ov[n * CO:(n + 1) * CO,
                               SB * B:SB * B + SB, u:u + 1, 0:2 * W],
                            stgv[64 * u:64 * u + 64, :,
                                 n * CPI:(n + 1) * CPI])

    nc.finalize()
    return nc


def prep_weights(weight: np.ndarray) -> dict:
    """Host-side prep of the tiny replicated weight tensor: quantize
    (identical fp32 math to the reference) and lay out as the matmul
    stationary [(4 image-groups x 32 ci), (9 taps x 64 co)] in fp16."""
    w = weight.astype(np.float32)
    amax_w = np.float32(np.max(np.abs(w)))
    sw = np.float32(127.0) / amax_w
    qw = np.round(w * sw)  # RNE, matches jnp.round; |qw| <= 127 exact in fp16
    qs = np.transpose(qw.reshape(64, 32, 9), (1, 2, 0)).reshape(32, 576)
    qstat = np.ascontiguousarray(np.tile(qs, (4, 1))).astype(np.float16)
    swv = np.full((128, 1), sw, np.float32)
    return {"qw_stat": qstat, "swv": swv}


def kernel(x: np.ndarray, weight: np.ndarray, bias: np.ndarray) -> np.ndarray:
    from concourse.bass_utils import run_bass_kernel_spmd

    n_cores = 8
    N = x.shape[0]
    per = N // n_cores
    nc = build(nimg=per, H=x.shape[2], W=x.shape[3], n_cores=n_cores)
    wp = prep_weights(np.asarray(weight))
    in_maps = [
        {
            "x": np.ascontiguousarray(x[i * per:(i + 1) * per]),
            "qw_stat": wp["qw_stat"],
            "swv": wp["swv"],
            "bias": np.ascontiguousarray(bias),
        }
        for i in range(n_cores)
    ]
    res = run_bass_kernel_spmd(nc, in_maps, core_ids=list(range(n_cores)))
    outs = [np.asarray(r["out"]).astype(np.float32) for r in res.results]
    return np.concatenate(outs, axis=0)


if __name__ == "__main__":
    # smoke: tiny build only
    nc = build(nimg=4, H=8, W=8, n_cores=2)
    print("build ok")
